# revision 26
# baseline (speedup 1.0000x reference)
"""Trainium2 Bass kernel for the gnn_message_passing attention problem.

Math (per sample b of B=2048):
    numN = 1 + count(L[b,0,:] >= 1)
    Q,K,V = tanh(A{q,k,v} @ x_b)          # [16,256] each, contraction over d=6
    S     = (Q @ K^T) / sqrt(numN)        # [16,16]
    P     = softmax(S, axis=1)
    o     = P @ V                         # [16,256]
    H     = silu(Ao @ o)                  # [256]
    out_b = (sum H)^2

Only row 0 of L is read -> host slices L[:,0,:] (2MB of the 537MB input).

Device mapping (per core, 256 samples = 32 groups of 8 = 4 quads):
  * 8 samples x 16 heads = 128 partitions per "group".
  * QK projections of a pair of groups land in one 2-bank PSUM tile ->
    ONE tanh per pair; V projections in a 1-bank tile -> one more.
    (ACT overhead is ~222 cycles/op, so fewer/bigger ACT ops win.)
  * Scores: per-group Q^T K with exact fp32 matmuls (free dim 128 costs
    4 cyc/row for fp32 and fp32r alike).
  * softmax: 1/sqrt(numN) scale + additive -1e9 block-diag mask fused in
    one DVE scalar_tensor_tensor per group; ONE exp per pair; denominator
    via a DVE chunked reduce; the division is folded into the block-diag
    Ao operand of the w^T matmul.
  * z accumulation: 8 groups' w^T vectors -> zero-padded block lhsT,
    8 accumulating matmuls into a [64,256] PSUM tile, one DVE copy out.
  * All small constants ride in ONE packed DMA (each dma_start costs
    ~625ns serially on the HWDGE); x loads are 4 quad-sized DMAs.
  * silu via exp + DVE reciprocal (everything stays inside the
    exp_and_others ACT table set; rsqrt(numN) is DVE Newton iteration).
"""

import numpy as np

import concourse.bass as bass
import concourse.bacc as bacc
import concourse.mybir as mybir
import concourse.tile as tile
from concourse.bass_utils import run_bass_kernel_spmd

F32 = mybir.dt.float32
F32R = mybir.dt.float32r
I32 = mybir.dt.int32
AF = mybir.ActivationFunctionType
OP = mybir.AluOpType

B, D, N, R = 2048, 6, 256, 16
NCORES = 8
BL = B // NCORES          # 256 samples per core
SPG = 8                   # samples per group
G = BL // SPG             # 32 groups
NPAIR = G // 2            # 16
NQUAD = G // 8            # 4
KD = SPG * D              # 48 = block-diag contraction dim
TW = 128
NEG = -1.0e9
RSQRT_MAGIC = 0x5F3759DF

# packed const layout: [128, 648]
#   abdqk [48,256] @0, abdv [48,128] @256, aobd [128,8] @384,
#   mask1 [128,128] @392, repmat [8,128] @520
CW = 648

# fp32r (1 cyc/row when free>=256, ~tf32 product precision) for the
# projection and z matmuls; exact fp32 for scores / w^T. The verifier
# requires every producer of an fp32r matmul operand to emit fp32r.
USE_F32R = True
DT_X = F32R if USE_F32R else F32

# bisect flags (debug)
import os
SKIP_Z = os.environ.get("KP_SKIP_Z") == "1"
SKIP_FINAL = os.environ.get("KP_SKIP_FINAL") == "1"
PLAIN_OUT = os.environ.get("KP_PLAIN_OUT") == "1"


def build_nc():
    nc = bacc.Bacc()

    xbd = nc.dram_tensor("xbd", [KD, G, N], DT_X, kind="ExternalInput")
    lrow = nc.dram_tensor("lrow", [TW, 2, N], F32, kind="ExternalInput")
    cpack = nc.dram_tensor("cpack", [TW, CW], DT_X, kind="ExternalInput")
    out_t = nc.dram_tensor("out", [BL], F32, kind="ExternalOutput")
    sinvs = nc.dram_tensor("sinvs", [BL], F32, kind="Internal")

    with tile.TileContext(nc) as tc:
        _body(tc, xbd, lrow, cpack, out_t, sinvs)
    nc.compile()
    return nc


def _body(tc, xbd, lrow, cpack, out_t, sinvs):
    from contextlib import ExitStack

    nc = tc.nc
    ctx = ExitStack()
    consts = ctx.enter_context(tc.tile_pool(name="consts", bufs=1))
    work = ctx.enter_context(tc.tile_pool(name="work", bufs=3))
    pairp = ctx.enter_context(tc.tile_pool(name="pairp", bufs=2))
    xinp = ctx.enter_context(tc.tile_pool(name="xinp", bufs=3))
    qktp = ctx.enter_context(tc.tile_pool(name="qktp", bufs=3))
    vtp = ctx.enter_context(tc.tile_pool(name="vtp", bufs=7))
    ps_qk = ctx.enter_context(tc.tile_pool(name="ps_qk", bufs=2, space="PSUM"))
    ps_v = ctx.enter_context(tc.tile_pool(name="ps_v", bufs=2, space="PSUM"))
    ps_sc = ctx.enter_context(tc.tile_pool(name="ps_sc", bufs=1, space="PSUM"))
    ps_wz = ctx.enter_context(tc.tile_pool(name="ps_wz", bufs=1, space="PSUM"))

    # ---- input DMAs. Each dma_start costs ~650ns on the in-order SP
    # sequencer / HWDGE: lrow first (it feeds the long numN chain), then
    # the constants pack that gates the first projections.
    c_lrow = consts.tile([TW, 2, N], F32)
    nc.sync.dma_start(out=c_lrow, in_=lrow[:, :, :])

    cc = consts.tile([TW, CW], DT_X)
    nc.sync.dma_start(out=cc, in_=cpack[:, :])
    c_abdqk = cc[0:KD, 0:256]
    c_abdv = cc[0:KD, 256:384]
    c_aobd = cc[:, 384:392].bitcast(F32)
    c_mask1 = cc[:, 392:520].bitcast(F32)
    c_repmat = cc[0:SPG, 520:648].bitcast(F32)

    # persistent zero-padded block-diagonal w^T staging tiles (two slots,
    # zeroed once; only the diagonal 8x8 blocks are ever rewritten)
    wtzs = []
    for s in range(2):
        wtz = consts.tile([TW, SPG, 8 * SPG], DT_X, name=f"wtz{s}", tag=f"wtz{s}")
        nc.gpsimd.memset(wtz.bitcast(mybir.dt.uint32), 0)
        wtzs.append(wtz)

    # ---- numN -> 1/sqrt(numN), replicated to (b,r) partition layout ----
    cnt = consts.tile([TW, 2], F32)
    junk = consts.tile([TW, N], F32)
    for ch in range(2):
        nc.vector.tensor_scalar(
            out=junk,
            in0=c_lrow[:, ch, :],
            scalar1=1.0,
            scalar2=0.0,
            op0=OP.is_ge,
            op1=OP.add,
            accum_out=cnt[:, ch : ch + 1],
        )
    m_t = consts.tile([TW, 2], F32)
    nc.vector.tensor_scalar(
        out=m_t, in0=cnt, scalar1=1.0, scalar2=None, op0=OP.add
    )
    # rsqrt via bit trick + 3 Newton iterations (DVE only, no ACT table)
    y_t = consts.tile([TW, 2], F32)
    t_t = consts.tile([TW, 2], F32)
    mi = m_t.bitcast(I32)
    yi = y_t.bitcast(I32)
    nc.vector.tensor_scalar(
        out=yi, in0=mi, scalar1=1, scalar2=None, op0=OP.arith_shift_right
    )
    nc.vector.tensor_scalar(
        out=yi, in0=yi, scalar1=-1, scalar2=RSQRT_MAGIC, op0=OP.mult, op1=OP.add
    )
    for _ in range(3):
        nc.vector.tensor_tensor(out=t_t, in0=y_t, in1=y_t, op=OP.mult)
        nc.vector.tensor_tensor(out=t_t, in0=t_t, in1=m_t, op=OP.mult)
        nc.vector.tensor_scalar(
            out=t_t, in0=t_t, scalar1=-0.5, scalar2=1.5, op0=OP.mult, op1=OP.add
        )
        nc.vector.tensor_tensor(out=y_t, in0=y_t, in1=t_t, op=OP.mult)

    # replicate invs[sample] -> [16b+r, g] via DRAM bounce + PE broadcast
    # (SWDGE queue so the waiting descriptors never block the HWDGE)
    nc.gpsimd.dma_start(
        out=bass.AP(sinvs, 0, [[1, TW], [TW, 2]]),
        in_=y_t[:, :],
    )
    invsT = consts.tile([SPG, G], F32)
    nc.gpsimd.dma_start(
        out=invsT,
        in_=bass.AP(sinvs, 0, [[1, SPG], [SPG, G]]),
    )
    rep_ps = ps_wz.tile([TW, G], F32, tag="wz")
    nc.tensor.matmul(rep_ps, c_repmat, invsT, start=True, stop=True)
    repinvs = consts.tile([TW, G], F32)
    nc.vector.tensor_copy(repinvs, rep_ps)

    zbuf = consts.tile([TW, 2, N], F32)
    sums = consts.tile([TW, 2], F32)

    qkvts = {}      # pair index -> qkt tile
    vts = {}        # pair index -> vt tile
    wtp8s = {}      # quad index -> wtp8 psum tile
    xins = {}

    def stage_a(i):
        """projections + tanh of pair i"""
        q, ip = divmod(i, 4)
        if ip == 0:
            for half in range(2):
                xin = xinp.tile(
                    [KD, 4, N], DT_X, tag=f"xin{half}", name=f"xin{half}_{q}"
                )
                nc.sync.dma_start(
                    out=xin,
                    in_=xbd[:, SPG * q + 4 * half : SPG * q + 4 * (half + 1), :],
                )
                xins.setdefault(q, []).append(xin)
        xin = xins[q][ip // 2]

        # QK projections (transposed layout) of the pair: 2-bank tile
        qkp = ps_qk.tile([TW, 4, N], F32, tag="qkp", name=f"qkp_{i}")
        for j in range(2):
            for h in range(2):
                nc.tensor.matmul(
                    qkp[:, 2 * j + h, :],
                    xin[:, 2 * (ip % 2) + j, h * TW : (h + 1) * TW],
                    c_abdqk,
                    start=True,
                    stop=True,
                )
        qkt = qktp.tile([TW, 4, N], F32, tag="qkt", name=f"qkt_{i}")
        nc.scalar.activation(qkt, qkp, AF.Tanh)
        qkvts[i] = qkt

        # V projections (direct layout): 1-bank tile; tanh emits fp32r so
        # the z matmuls can take the fp32r fast path
        vp = ps_v.tile([TW, 2, N], F32, tag="vp", name=f"vp_{i}")
        for j in range(2):
            nc.tensor.matmul(
                vp[:, j, :],
                c_abdv,
                xin[:, 2 * (ip % 2) + j, :],
                start=True,
                stop=True,
            )
        vt = vtp.tile([TW, 2, N], DT_X, tag="vt", name=f"vt_{i}")
        nc.scalar.activation(vt, vp, AF.Tanh)
        vts[i] = vt

    def stage_b(i):
        """scores .. w^T of pair i (+ z batch at quad boundaries)"""
        q, ip = divmod(i, 4)
        g0 = 2 * i
        qkvt = qkvts[i]
        if ip == 0:
            wtp8s[q] = ps_wz.tile([TW, 8 * SPG], F32, tag="wz", name=f"wtp8_{q}")
        wtp8 = wtp8s[q]

        # scores: per-group Q^T K, exact fp32 (free=128)
        scp = ps_sc.tile([TW, 2, TW], F32, tag="scp", name=f"scp_{i}")
        for j in range(2):
            for h in range(2):
                nc.tensor.matmul(
                    scp[:, j, :],
                    qkvt[:, 2 * j + h, 0:TW],
                    qkvt[:, 2 * j + h, TW : 2 * TW],
                    start=(h == 0),
                    stop=(h == 1),
                )
        # scale by 1/sqrt(numN) and add -1e9 off-block mask
        scm = pairp.tile([TW, 2, TW], F32, tag="scm", name=f"scm_{i}")
        for j in range(2):
            nc.vector.scalar_tensor_tensor(
                out=scm[:, j, :],
                in0=scp[:, j, :],
                scalar=repinvs[:, g0 + j : g0 + j + 1],
                in1=c_mask1,
                op0=OP.mult,
                op1=OP.add,
            )
        esb = pairp.tile([TW, 2, TW], F32, tag="esb", name=f"esb_{i}")
        nc.scalar.activation(esb, scm, AF.Exp)

        den = pairp.tile([TW, 2], F32, tag="den", name=f"den_{i}")
        nc.vector.tensor_reduce(
            out=den, in_=esb, axis=mybir.AxisListType.X, op=OP.add
        )
        rden = pairp.tile([TW, 2], F32, tag="rden", name=f"rden_{i}")
        nc.vector.reciprocal(out=rden, in_=den)

        for j in range(2):
            jq = 2 * ip + j
            aos = pairp.tile([TW, SPG], F32, tag=f"aos{j}", name=f"aos{j}_{i}")
            nc.vector.tensor_scalar(
                out=aos,
                in0=c_aobd,
                scalar1=rden[:, j : j + 1],
                scalar2=None,
                op0=OP.mult,
            )
            nc.tensor.matmul(
                wtp8[:, jq * SPG : (jq + 1) * SPG],
                esb[:, j, :],
                aos,
                start=True,
                stop=True,
            )

        if ip == 3 and not SKIP_Z:
            _quad_tail(q, wtp8)

    def _quad_tail(q, wtp8):
        wtz = wtzs[q % 2]
        # gather the 8 w^T column blocks onto wtz's block diagonal
        wtz_diag = bass.AP(
            tensor=wtz.tensor,
            offset=wtz.offset,
            ap=[[SPG * 8 * SPG, TW], [8 * SPG + SPG, SPG], [1, SPG]],
        )
        nc.vector.tensor_copy(wtz_diag, wtp8)

        zp8 = ps_wz.tile([64, N], F32, tag="wz", name=f"zp8_{q}")
        for jq in range(8):
            nc.tensor.matmul(
                zp8,
                wtz[:, jq, :],
                vts[4 * q + jq // 2][:, jq % 2, :],
                start=(jq == 0),
                stop=(jq == 7),
            )
        pbase = 64 * (q % 2)
        ch = q // 2
        nc.vector.tensor_copy(zbuf[pbase : pbase + 64, ch, :], zp8)

        # finalize each zbuf chunk as soon as its 16 groups are done
        if q % 2 == 1 and not SKIP_FINAL:
            sig = work.tile([TW, N], F32, tag="sig", name=f"sig_{q}")
            nc.scalar.activation(sig, zbuf[:, ch, :], AF.Exp, scale=-1.0)
            nc.vector.tensor_scalar(
                out=sig, in0=sig, scalar1=1.0, scalar2=None, op0=OP.add
            )
            nc.vector.reciprocal(out=sig, in_=sig)
            hbuf = work.tile([TW, N], F32, tag="hbuf", name=f"hbuf_{q}")
            nc.vector.tensor_tensor(
                out=hbuf, in0=zbuf[:, ch, :], in1=sig, op=OP.mult
            )
            nc.vector.tensor_reduce(
                out=sums[:, ch : ch + 1],
                in_=hbuf,
                axis=mybir.AxisListType.X,
                op=OP.add,
            )

    # ---- software-pipelined main loop (depth 1) ----
    for i in range(NPAIR + 1):
        if i < NPAIR:
            stage_a(i)
        if i > 0:
            stage_b(i - 1)

    if SKIP_Z or SKIP_FINAL:
        nc.vector.memset(sums, 0.0)
    sqs = consts.tile([TW, 2], F32)
    nc.vector.tensor_tensor(out=sqs, in0=sums, in1=sums, op=OP.mult)
    if PLAIN_OUT:
        nc.sync.dma_start(out=bass.AP(out_t, 0, [[1, TW * 2]]), in_=sqs[:, :])
    else:
        nc.sync.dma_start(
            out=bass.AP(out_t, 0, [[1, TW], [TW, 2]]),
            in_=sqs[:, :],
        )

    ctx.close()


# ---------------------------------------------------------------------------
# host-side packing


def _prep_consts(Aq, Ak, Av, Ao):
    cpack = np.zeros((TW, CW), np.float32)
    for b in range(SPG):
        cpack[b * D : (b + 1) * D, b * R : (b + 1) * R] = Aq.T
        cpack[b * D : (b + 1) * D, TW + b * R : TW + (b + 1) * R] = Ak.T
        cpack[b * D : (b + 1) * D, 256 + b * R : 256 + (b + 1) * R] = Av.T
        cpack[b * R : (b + 1) * R, 384 + b] = Ao[0, :]
        cpack[b, 520 + b * R : 520 + (b + 1) * R] = 1.0
    m = np.full((TW, TW), NEG, np.float32)
    for b in range(SPG):
        m[b * R : (b + 1) * R, b * R : (b + 1) * R] = 0.0
    cpack[:, 392:520] = m
    return cpack


def _prep_core(x_c, L_c):
    # x_c [256, 6, 256] -> xbd [48, 32, 256] : xbd[b*6+d, g, n] = x[8g+b, d, n]
    xbd = np.ascontiguousarray(
        x_c.reshape(G, SPG, D, N).transpose(1, 2, 0, 3).reshape(KD, G, N)
    )
    # L_c [256, 256] (row 0 of L) -> lrow [128, 2, 256]
    lrow = np.ascontiguousarray(L_c.reshape(2, TW, N).transpose(1, 0, 2))
    return xbd, lrow


_NC_CACHE = {}


def _get_nc():
    if "nc" not in _NC_CACHE:
        _NC_CACHE["nc"] = build_nc()
    return _NC_CACHE["nc"]


def _make_in_maps(x, L, Aq, Ak, Av, Ao):
    x = np.asarray(x, np.float32)
    Lrow = np.ascontiguousarray(np.asarray(L)[:, 0, :], dtype=np.float32)
    cpack = _prep_consts(
        np.asarray(Aq, np.float32),
        np.asarray(Ak, np.float32),
        np.asarray(Av, np.float32),
        np.asarray(Ao, np.float32),
    )
    in_maps = []
    for c in range(NCORES):
        xbd, lrow = _prep_core(x[c * BL : (c + 1) * BL], Lrow[c * BL : (c + 1) * BL])
        in_maps.append(dict(xbd=xbd, lrow=lrow, cpack=cpack))
    return in_maps


def kernel(x, L, Aq, Ak, Av, Ao):
    in_maps = _make_in_maps(x, L, Aq, Ak, Av, Ao)
    nc = _get_nc()
    res = run_bass_kernel_spmd(nc, in_maps, core_ids=list(range(NCORES)))
    out = np.concatenate(
        [res.results[c]["out"].reshape(BL) for c in range(NCORES)]
    )
    return out.reshape(B, 1).astype(np.float32)


# exposed for test.py
def run_profiled(x, L, Aq, Ak, Av, Ao, **kw):
    in_maps = _make_in_maps(x, L, Aq, Ak, Av, Ao)
    nc = _get_nc()
    res = run_bass_kernel_spmd(nc, in_maps, core_ids=list(range(NCORES)), **kw)
    out = np.concatenate([res.results[c]["out"].reshape(BL) for c in range(NCORES)])
    return out.reshape(B, 1).astype(np.float32), res


# revision 40
# speedup vs baseline: 174.1312x; 174.1312x over previous
"""Trainium2 Bass kernel for the gnn_message_passing attention problem.

Math (per sample b of B=2048):
    numN = 1 + count(L[b,0,:] >= 1)
    Q,K,V = tanh(A{q,k,v} @ x_b)          # [16,256] each, contraction over d=6
    S     = (Q @ K^T) / sqrt(numN)        # [16,16]
    P     = softmax(S, axis=1)
    o     = P @ V                         # [16,256]
    H     = silu(Ao @ o)                  # [256]
    out_b = (sum H)^2

Only row 0 of L is read -> host slices L[:,0,:] (2MB of the 537MB input).

Device mapping (per core, 256 samples = 32 groups of 8 = 4 quads):
  * 8 samples x 16 heads = 128 partitions per "group".
  * QK projections of a pair of groups land in one 2-bank PSUM tile ->
    ONE tanh per pair; V projections in a 1-bank tile -> one more.
    (ACT overhead is ~222 cycles/op, so fewer/bigger ACT ops win.)
  * Scores: per-group Q^T K with exact fp32 matmuls (free dim 128 costs
    4 cyc/row for fp32 and fp32r alike).
  * softmax: 1/sqrt(numN) scale + additive -1e9 block-diag mask fused in
    one DVE scalar_tensor_tensor per group; ONE exp per pair; denominator
    via a DVE chunked reduce; the division is folded into the block-diag
    Ao operand of the w^T matmul.
  * z accumulation: 8 groups' w^T vectors -> zero-padded block lhsT,
    8 accumulating matmuls into a [64,256] PSUM tile, one DVE copy out.
  * All small constants ride in ONE packed DMA (each dma_start costs
    ~625ns serially on the HWDGE); x loads are 4 quad-sized DMAs.
  * silu via exp + DVE reciprocal (everything stays inside the
    exp_and_others ACT table set; rsqrt(numN) is DVE Newton iteration).
"""

import numpy as np

import concourse.bass as bass
import concourse.bacc as bacc
import concourse.mybir as mybir
import concourse.tile as tile
from concourse.bass_utils import run_bass_kernel_spmd

F32 = mybir.dt.float32
F32R = mybir.dt.float32r
I32 = mybir.dt.int32
AF = mybir.ActivationFunctionType
OP = mybir.AluOpType

B, D, N, R = 2048, 6, 256, 16
NCORES = 8
BL = B // NCORES          # 256 samples per core
SPG = 8                   # samples per group
G = BL // SPG             # 32 groups
NPAIR = G // 2            # 16
NQUAD = G // 8            # 4
KD = SPG * D              # 48 = block-diag contraction dim
TW = 128
NEG = -1.0e9
RSQRT_MAGIC = 0x5F3759DF

# packed const layout: [128, 648]
#   abdqk [48,256] @0, abdv [48,128] @256, aobd [128,8] @384,
#   mask1 [128,128] @392, repmat [8,128] @520
CW = 648

# fp32r (1 cyc/row when free>=256, ~tf32 product precision) for the
# projection and z matmuls; exact fp32 for scores / w^T. The verifier
# requires every producer of an fp32r matmul operand to emit fp32r.
USE_F32R = True
DT_X = F32R if USE_F32R else F32

# bisect flags (debug)
import os
SKIP_Z = os.environ.get("KP_SKIP_Z") == "1"
SKIP_FINAL = os.environ.get("KP_SKIP_FINAL") == "1"
PLAIN_OUT = os.environ.get("KP_PLAIN_OUT") == "1"
NO_GPSIMD = os.environ.get("KP_NO_GPSIMD", "1") == "1"
DUMP = os.environ.get("KP_DUMP") == "1"



def build_nc():
    nc = bacc.Bacc()

    xbd = nc.dram_tensor("xbd", [KD, G, N], DT_X, kind="ExternalInput")
    lrow = nc.dram_tensor("lrow", [TW, 2, N], F32, kind="ExternalInput")
    cpack = nc.dram_tensor("cpack", [TW, CW], DT_X, kind="ExternalInput")
    out_t = nc.dram_tensor("out", [BL], F32, kind="ExternalOutput")
    dbg = (
        nc.dram_tensor("zdump", [TW, 2, N], F32, kind="ExternalOutput"),
        nc.dram_tensor("hdump", [TW, 2, N], F32, kind="ExternalOutput"),
        nc.dram_tensor("scdump", [TW, 2, TW], F32, kind="ExternalOutput"),
        nc.dram_tensor("esdump", [TW, 2, TW], F32, kind="ExternalOutput"),
        nc.dram_tensor("wtdump", [TW, 8 * SPG], F32, kind="ExternalOutput"),
        nc.dram_tensor("qktdump", [TW, 4, N], F32, kind="ExternalOutput"),
    ) if DUMP else None
    sinvs = nc.dram_tensor("sinvs", [BL], F32, kind="Internal")

    with tile.TileContext(nc) as tc:
        _body(tc, xbd, lrow, cpack, out_t, sinvs, dbg)
    nc.compile()
    return nc


def _body(tc, xbd, lrow, cpack, out_t, sinvs, dbg=None):
    from contextlib import ExitStack

    nc = tc.nc
    ctx = ExitStack()
    consts = ctx.enter_context(tc.tile_pool(name="consts", bufs=1))
    work = ctx.enter_context(tc.tile_pool(name="work", bufs=3))
    pairp = ctx.enter_context(tc.tile_pool(name="pairp", bufs=2))
    xinp = ctx.enter_context(tc.tile_pool(name="xinp", bufs=3))
    qktp = ctx.enter_context(tc.tile_pool(name="qktp", bufs=3))
    vtp = ctx.enter_context(tc.tile_pool(name="vtp", bufs=7))
    ps_qk = ctx.enter_context(tc.tile_pool(name="ps_qk", bufs=2, space="PSUM"))
    ps_v = ctx.enter_context(tc.tile_pool(name="ps_v", bufs=2, space="PSUM"))
    ps_sc = ctx.enter_context(tc.tile_pool(name="ps_sc", bufs=1, space="PSUM"))
    ps_wz = ctx.enter_context(tc.tile_pool(name="ps_wz", bufs=1, space="PSUM"))

    # ---- input DMAs. Each dma_start costs ~650ns on the in-order SP
    # sequencer / HWDGE; order trades first-projection start vs the
    # numN-chain (lrow) critical path.
    import os as _os
    _lrow_first = _os.environ.get("KP_LROW_FIRST", "1") == "1"
    c_lrow = consts.tile([TW, 2, N], F32)
    cc = consts.tile([TW, CW], DT_X)
    if _lrow_first:
        nc.sync.dma_start(out=c_lrow, in_=lrow[:, :, :])
        nc.sync.dma_start(out=cc, in_=cpack[:, :])
    else:
        nc.sync.dma_start(out=cc, in_=cpack[:, :])
        nc.sync.dma_start(out=c_lrow, in_=lrow[:, :, :])
    c_abdqk = cc[0:KD, 0:256]
    c_abdv = cc[0:KD, 256:384]
    c_aobd = cc[:, 384:392].bitcast(F32)
    c_mask1 = cc[:, 392:520].bitcast(F32)
    c_repmat = cc[0:SPG, 520:648].bitcast(F32)

    # persistent zero-padded block-diagonal w^T staging tiles (two slots,
    # zeroed once; only the diagonal 8x8 blocks are ever rewritten)
    wtzs = []
    for s in range(2):
        wtz = consts.tile([TW, SPG, 8 * SPG], DT_X, name=f"wtz{s}", tag=f"wtz{s}")
        if NO_GPSIMD:
            nc.vector.memset(wtz.bitcast(mybir.dt.uint32), 0)
        else:
            nc.gpsimd.memset(wtz.bitcast(mybir.dt.uint32), 0)
        wtzs.append(wtz)

    zbuf = consts.tile([TW, 2, N], F32)
    sums = consts.tile([TW, 2], F32)

    qkvts = {}      # pair index -> qkt tile
    vts = {}        # pair index -> vt tile
    wtp8s = {}      # quad index -> wtp8 psum tile
    xins = {}

    def stage_a(i):
        """projections + tanh of pair i"""
        q, ip = divmod(i, 4)
        if ip == 0:
            for half in range(2):
                xin = xinp.tile(
                    [KD, 4, N], DT_X, tag=f"xin{half}", name=f"xin{half}_{q}"
                )
                nc.sync.dma_start(
                    out=xin,
                    in_=xbd[:, SPG * q + 4 * half : SPG * q + 4 * (half + 1), :],
                )
                xins.setdefault(q, []).append(xin)
        xin = xins[q][ip // 2]

        # QK projections (transposed layout) of the pair: 2-bank tile
        qkp = ps_qk.tile([TW, 4, N], F32, tag="qkp", name=f"qkp_{i}")
        for j in range(2):
            for h in range(2):
                nc.tensor.matmul(
                    qkp[:, 2 * j + h, :],
                    xin[:, 2 * (ip % 2) + j, h * TW : (h + 1) * TW],
                    c_abdqk,
                    start=True,
                    stop=True,
                )
        qkt = qktp.tile([TW, 4, N], F32, tag="qkt", name=f"qkt_{i}")
        nc.scalar.activation(qkt, qkp, AF.Tanh)
        if dbg is not None and i == 0:
            nc.sync.dma_start(out=dbg[5][:, :, :], in_=qkt)
        qkvts[i] = qkt

        # V projections (direct layout): 1-bank tile; tanh emits fp32r so
        # the z matmuls can take the fp32r fast path
        vp = ps_v.tile([TW, 2, N], F32, tag="vp", name=f"vp_{i}")
        for j in range(2):
            nc.tensor.matmul(
                vp[:, j, :], c_abdv, xin[:, 2 * (ip % 2) + j, :],
                start=True, stop=True,
            )
        vt = vtp.tile([TW, 2, N], DT_X, tag="vt", name=f"vt_{i}")
        nc.scalar.activation(vt, vp, AF.Tanh)
        vts[i] = vt

    def stage_b(i):
        """scores .. w^T of pair i (+ z batch at quad boundaries)"""
        q, ip = divmod(i, 4)
        g0 = 2 * i
        qkvt = qkvts[i]
        if ip == 0:
            wtp8s[q] = ps_wz.tile([TW, 8 * SPG], F32, tag="wz", name=f"wtp8_{q}")
        wtp8 = wtp8s[q]

        # scores: per-group Q^T K, exact fp32 (free=128)
        scp = ps_sc.tile([TW, 2, TW], F32, tag="scp", name=f"scp_{i}")
        for j in range(2):
            for h in range(2):
                nc.tensor.matmul(
                    scp[:, j, :],
                    qkvt[:, 2 * j + h, 0:TW],
                    qkvt[:, 2 * j + h, TW : 2 * TW],
                    start=(h == 0),
                    stop=(h == 1),
                )
        # scale by 1/sqrt(numN) and add -1e9 off-block mask
        scm = pairp.tile([TW, 2, TW], F32, tag="scm", name=f"scm_{i}")
        for j in range(2):
            nc.vector.scalar_tensor_tensor(
                out=scm[:, j, :],
                in0=scp[:, j, :],
                scalar=repinvs[:, g0 + j : g0 + j + 1],
                in1=c_mask1,
                op0=OP.mult,
                op1=OP.add,
            )
        esb = pairp.tile([TW, 2, TW], F32, tag="esb", name=f"esb_{i}")
        nc.scalar.activation(esb, scm, AF.Exp)
        if dbg is not None and i == 0:
            nc.sync.dma_start(out=dbg[2][:, :, :], in_=scm)
            nc.sync.dma_start(out=dbg[3][:, :, :], in_=esb)

        den = pairp.tile([TW, 2], F32, tag="den", name=f"den_{i}")
        nc.vector.tensor_reduce(
            out=den, in_=esb, axis=mybir.AxisListType.X, op=OP.add
        )
        rden = pairp.tile([TW, 2], F32, tag="rden", name=f"rden_{i}")
        nc.vector.reciprocal(out=rden, in_=den)

        for j in range(2):
            jq = 2 * ip + j
            aos = pairp.tile([TW, SPG], F32, tag=f"aos{j}", name=f"aos{j}_{i}")
            nc.vector.tensor_scalar(
                out=aos,
                in0=c_aobd,
                scalar1=rden[:, j : j + 1],
                scalar2=None,
                op0=OP.mult,
            )
            nc.tensor.matmul(
                wtp8[:, jq * SPG : (jq + 1) * SPG],
                esb[:, j, :],
                aos,
                start=True,
                stop=True,
            )

        if ip == 3 and not SKIP_Z:
            _quad_tail(q, wtp8)

    def _quad_tail(q, wtp8):
        wtz = wtzs[q % 2]
        # gather the 8 w^T column blocks onto wtz's block diagonal
        wtz_diag = bass.AP(
            tensor=wtz.tensor,
            offset=wtz.offset,
            ap=[[SPG * 8 * SPG, TW], [8 * SPG + SPG, SPG], [1, SPG]],
        )
        nc.vector.tensor_copy(wtz_diag, wtp8)
        if dbg is not None and q == 0:
            nc.sync.dma_start(
                out=dbg[4][:, :], in_=wtz.bitcast(F32).rearrange("p a b -> p (a b)")[:, 0 : 8 * SPG]
            )

        zp8 = ps_wz.tile([64, N], F32, tag="wz", name=f"zp8_{q}")
        for jq in range(8):
            nc.tensor.matmul(
                zp8,
                wtz[:, jq, :],
                vts[4 * q + jq // 2][:, jq % 2, :],
                start=(jq == 0),
                stop=(jq == 7),
            )
        pbase = 64 * (q % 2)
        ch = q // 2
        nc.vector.tensor_copy(zbuf[pbase : pbase + 64, ch, :], zp8)

        # finalize each zbuf chunk as soon as its 16 groups are done
        if q % 2 == 1 and not SKIP_FINAL:
            sig = work.tile([TW, N], F32, tag="sig", name=f"sig_{q}")
            nc.scalar.activation(sig, zbuf[:, ch, :], AF.Exp, scale=-1.0)
            nc.vector.tensor_scalar(
                out=sig, in0=sig, scalar1=1.0, scalar2=None, op0=OP.add
            )
            nc.vector.reciprocal(out=sig, in_=sig)
            hbuf = work.tile([TW, N], F32, tag="hbuf", name=f"hbuf_{q}")
            nc.vector.scalar_tensor_tensor(
                out=hbuf,
                in0=zbuf[:, ch, :],
                scalar=1.0,
                in1=sig,
                op0=OP.mult,
                op1=OP.mult,
                accum_out=sums[:, ch : ch + 1],
            )
            if dbg is not None:
                nc.sync.dma_start(out=dbg[0][:, ch, :], in_=zbuf[:, ch, :])
                nc.sync.dma_start(out=dbg[1][:, ch, :], in_=hbuf)

    # prime the pipeline: two pairs of projections first so their x DMA
    # descriptors beat the (Newton-blocked) sinvs DMAs into the SP queue
    stage_a(0)
    stage_a(1)

    # ---- numN -> 1/sqrt(numN), replicated to (b,r) partition layout ----
    cnt = consts.tile([TW, 2], F32)
    junk = consts.tile([TW, N], F32)
    for ch in range(2):
        nc.vector.tensor_scalar(
            out=junk,
            in0=c_lrow[:, ch, :],
            scalar1=1.0,
            scalar2=0.0,
            op0=OP.is_ge,
            op1=OP.add,
            accum_out=cnt[:, ch : ch + 1],
        )
    m_t = consts.tile([TW, 2], F32)
    nc.vector.tensor_scalar(
        out=m_t, in0=cnt, scalar1=1.0, scalar2=None, op0=OP.add
    )
    # rsqrt via bit trick + 3 Newton iterations (DVE only, no ACT table)
    y_t = consts.tile([TW, 2], F32)
    t_t = consts.tile([TW, 2], F32)
    mi = m_t.bitcast(I32)
    yi = y_t.bitcast(I32)
    nc.vector.tensor_scalar(
        out=yi, in0=mi, scalar1=1, scalar2=None, op0=OP.arith_shift_right
    )
    nc.vector.tensor_scalar(
        out=yi, in0=yi, scalar1=-1, scalar2=RSQRT_MAGIC, op0=OP.mult, op1=OP.add
    )
    for _ in range(3):
        nc.vector.tensor_tensor(out=t_t, in0=y_t, in1=y_t, op=OP.mult)
        nc.vector.tensor_tensor(out=t_t, in0=t_t, in1=m_t, op=OP.mult)
        nc.vector.tensor_scalar(
            out=t_t, in0=t_t, scalar1=-0.5, scalar2=1.5, op0=OP.mult, op1=OP.add
        )
        nc.vector.tensor_tensor(out=y_t, in0=y_t, in1=t_t, op=OP.mult)

    # replicate invs[sample] -> [16b+r, g] via DRAM bounce + PE broadcast
    dma_eng = nc.sync if NO_GPSIMD else nc.gpsimd
    dma_eng.dma_start(
        out=bass.AP(sinvs, 0, [[1, TW], [TW, 2]]),
        in_=y_t[:, :],
    )
    invsT = consts.tile([SPG, G], F32)
    dma_eng.dma_start(
        out=invsT,
        in_=bass.AP(sinvs, 0, [[1, SPG], [SPG, G]]),
    )
    rep_ps = ps_wz.tile([TW, G], F32, tag="wz")
    nc.tensor.matmul(rep_ps, c_repmat, invsT, start=True, stop=True)
    repinvs = consts.tile([TW, G], F32)
    nc.vector.tensor_copy(repinvs, rep_ps)


    # ---- software-pipelined main loop (depth 2 during warmup) ----
    for i in range(2, NPAIR + 2):
        if i < NPAIR:
            stage_a(i)
        stage_b(i - 2)

    if SKIP_Z or SKIP_FINAL:
        nc.vector.memset(sums, 0.0)
    sqs = consts.tile([TW, 2], F32)
    nc.vector.tensor_tensor(out=sqs, in0=sums, in1=sums, op=OP.mult)
    if PLAIN_OUT:
        nc.sync.dma_start(out=bass.AP(out_t, 0, [[1, TW * 2]]), in_=sqs[:, :])
    else:
        nc.sync.dma_start(
            out=bass.AP(out_t, 0, [[1, TW], [TW, 2]]),
            in_=sqs[:, :],
        )

    ctx.close()


# ---------------------------------------------------------------------------
# host-side packing


def _prep_consts(Aq, Ak, Av, Ao):
    cpack = np.zeros((TW, CW), np.float32)
    for b in range(SPG):
        cpack[b * D : (b + 1) * D, b * R : (b + 1) * R] = Aq.T
        cpack[b * D : (b + 1) * D, TW + b * R : TW + (b + 1) * R] = Ak.T
        cpack[b * D : (b + 1) * D, 256 + b * R : 256 + (b + 1) * R] = Av.T
        cpack[b * R : (b + 1) * R, 384 + b] = Ao[0, :]
        cpack[b, 520 + b * R : 520 + (b + 1) * R] = 1.0
    m = np.full((TW, TW), NEG, np.float32)
    for b in range(SPG):
        m[b * R : (b + 1) * R, b * R : (b + 1) * R] = 0.0
    cpack[:, 392:520] = m
    return cpack


def _prep_core(x_c, L_c):
    # x_c [256, 6, 256] -> xbd [48, 32, 256] : xbd[b*6+d, g, n] = x[8g+b, d, n]
    xbd = np.ascontiguousarray(
        x_c.reshape(G, SPG, D, N).transpose(1, 2, 0, 3).reshape(KD, G, N)
    )
    # L_c [256, 256] (row 0 of L) -> lrow [128, 2, 256]
    lrow = np.ascontiguousarray(L_c.reshape(2, TW, N).transpose(1, 0, 2))
    return xbd, lrow


_NC_CACHE = {}


def _get_nc():
    if "nc" not in _NC_CACHE:
        _NC_CACHE["nc"] = build_nc()
    return _NC_CACHE["nc"]


def _make_in_maps(x, L, Aq, Ak, Av, Ao):
    x = np.asarray(x, np.float32)
    Lrow = np.ascontiguousarray(np.asarray(L)[:, 0, :], dtype=np.float32)
    cpack = _prep_consts(
        np.asarray(Aq, np.float32),
        np.asarray(Ak, np.float32),
        np.asarray(Av, np.float32),
        np.asarray(Ao, np.float32),
    )
    in_maps = []
    for c in range(NCORES):
        xbd, lrow = _prep_core(x[c * BL : (c + 1) * BL], Lrow[c * BL : (c + 1) * BL])
        in_maps.append(dict(xbd=xbd, lrow=lrow, cpack=cpack))
    return in_maps


def kernel(x, L, Aq, Ak, Av, Ao):
    in_maps = _make_in_maps(x, L, Aq, Ak, Av, Ao)
    nc = _get_nc()
    res = run_bass_kernel_spmd(nc, in_maps, core_ids=list(range(NCORES)))
    out = np.concatenate(
        [res.results[c]["out"].reshape(BL) for c in range(NCORES)]
    )
    return out.reshape(B, 1).astype(np.float32)


# exposed for test.py
def run_profiled(x, L, Aq, Ak, Av, Ao, **kw):
    in_maps = _make_in_maps(x, L, Aq, Ak, Av, Ao)
    nc = _get_nc()
    res = run_bass_kernel_spmd(nc, in_maps, core_ids=list(range(NCORES)), **kw)
    out = np.concatenate([res.results[c]["out"].reshape(BL) for c in range(NCORES)])
    return out.reshape(B, 1).astype(np.float32), res
